# revision 1
# baseline (speedup 1.0000x reference)
"""Trainium2 Bass kernel for nn_Classifier (segment_reduce).

Computation (reference):
    local  = relu(x @ W1.T)            # [T, 50] @ [50, 400] -> [T, 400]
    feat   = mean over windows of J=24 # [T//24, 400]
    logits = feat @ W2.T               # [T//24, 400] @ [400, 10]

Strategy: pure data parallel over 8 NeuronCores (x sharded along T).
Per core (T_c = 98304 rows = 4096 windows):
  - Host packs the x shard TRANSPOSED + bf16 into xp [128, 49152]:
    rows 0-49 hold x_shard[:49152].T, rows 64-113 hold x_shard[49152:].T.
    This puts the contraction dim (n=50) on partitions so matmul1 needs no
    on-device transpose, and the two shard halves row-tile the PE array
    (tile_position (0,0) / (64,0)) for 2x concurrent matmuls.
  - matmul1: lhsT = xp[:, 128-col tile] (stationary), rhs = W1.T [50, 400]
    -> psum [128t, 400k] fp32; two tiles packed per [128, 1024] psum pair.
  - relu evacuation psum->sbuf bf16 split across ScalarE (Relu) and
    VectorE (tensor_scalar_max) — the throughput-limiting stage.
  - pooling runs on the PE: stationary 0/1 matrices contract 24-row
  - windows across psum partitions; 4 col-strips of the array accumulate
    into one shared feat psum bank per shard-half.
  - feat is transposed back with transpose-mode matmuls whose permutation
    operand also un-scrambles the window order, then matmul2 (W2.T/24)
    accumulates the logits, which DMA out in natural row order.
"""

import sys

sys.path.insert(0, "/opt/trn_rl_repo")

import numpy as np
import ml_dtypes

import bass_rust
import concourse.bass as bass
import concourse.mybir as mybir
import concourse.tile as tile
from concourse.bass_utils import run_bass_kernel_spmd
from concourse.tile import TileContext
from concourse.vector_clock import ScopedClock

# ---------------------------------------------------------------------------
# Wait-count legalization (monkeypatch).
#
# This walrus build accepts at most 1 sync-wait per instruction (2 for
# EventSemaphore), but Tile's scheduler and tail drain can attach more,
# failing codegen with "Too many sync wait commands". Spread excess waits
# onto same-engine NOPs inserted immediately before the instruction.
# ---------------------------------------------------------------------------

_orig_add = TileContext._add_instruction


def _wait_cap(inst):
    return 2 if type(inst).__name__ == "InstEventSemaphore" else 1


def _patched_add_instruction(self, inst):
    si = inst.sync_info
    cap = _wait_cap(inst)
    if (
        si is not None
        and si.on_wait
        and len(si.on_wait) > cap
        and inst.engine != mybir.EngineType.Unassigned
    ):
        waits = list(si.on_wait)
        for w in waits[:-cap]:
            nop = bass_rust.InstNoOp(
                name=f"I-waitfix-{self.nc.next_id()}",
                opcode="NoOp",
                engine=inst.engine,
                ins=[],
                outs=[],
            )
            nop.sync_info = mybir.SyncInfo(on_wait=[w], on_update=[])
            _orig_add(self, nop)
        inst.sync_info = mybir.SyncInfo(
            on_wait=waits[-cap:], on_update=list(si.on_update or [])
        )
    _orig_add(self, inst)


def _patched_drain_and_barrier(self, tick_clock, wait_clock):
    nc = self.nc
    drain_inst = nc.sync.drain()
    wait_clock.add_sem_waits(
        drain_inst.ins, ScopedClock({None: tick_clock.global_clock})
    )
    mi = drain_inst.ins
    si = mi.sync_info
    waits = list(si.on_wait) if (si and si.on_wait) else []
    if len(waits) > 1:
        mi.sync_info = mybir.SyncInfo(
            on_wait=[waits[-1]], on_update=list(si.on_update or [])
        )
        for w in waits[:-1]:
            nop = nc.sync.nop()
            nop.ins.sync_info = mybir.SyncInfo(on_wait=[w], on_update=[])

    nc.all_engine_barrier()
    assert self.sems is not None
    popped = nc._tile_sem_poison_stack.pop()
    assert popped is self._sem_poison
    nc.clear_and_free_semaphores(list(self.sems.allocated().values()))
    nc.all_engine_barrier()


TileContext._add_instruction = _patched_add_instruction
TileContext._drain_and_barrier = _patched_drain_and_barrier

# ---------------------------------------------------------------------------
# Problem constants (hardcoded per the harness contract)
# ---------------------------------------------------------------------------

J = 24
T, N, K, C = 786432, 50, 400, 10
NCORES = 8
TC = T // NCORES          # 98304 rows per core
H = TC // 2               # 49152 cols per half in xp
B_CORE = TC // J          # 4096 windows per core
NTILE = H // 128          # 384 tiles of 128 rows per half
NG = 16                   # supergroup iterations (8 groups x 3 tiles each)
CHUNK = 24 * 128          # 3072 xp columns per supergroup

BF16 = mybir.dt.bfloat16
F32 = mybir.dt.float32
nbf = ml_dtypes.bfloat16


def _build_pmats():
    """P[j, q] in one [128, 192] array: col block (j*2+q)*32.
    P[j,q][tau, 16*q + w] = 1 where window w = (128*j + tau) // 24 of the
    384-row group; q = which half of the 32-partition strip."""
    pm = np.zeros((128, 192), np.float32)
    for j_ in range(3):
        for q in range(2):
            base = (j_ * 2 + q) * 32
            for tau in range(128):
                w = (128 * j_ + tau) // 24
                pm[tau, base + 16 * q + w] = 1.0
    return pm.astype(nbf)


def _build_permmat():
    """Permutation for the feat transpose: featT column m takes feat row
    p = 32*s + 16*q + w where m = 64*q + 16*s + w (un-scrambles the
    pooling strip layout into natural window order)."""
    pm = np.zeros((128, 128), np.float32)
    for m in range(128):
        w = m % 16
        s = (m // 16) % 4
        q = m // 64
        p = 32 * s + 16 * q + w
        pm[p, m] = 1.0
    return pm.astype(nbf)


def _build_nc(repeat: int = 1):
    """repeat>1 re-runs the whole computation in one NEFF — used by the
    test harness to measure device time differentially (wall(R=3) -
    wall(R=1))/2 without NTFF profiling."""
    nc = bass.Bass()
    xp_d = nc.declare_dram_parameter("xp", [128, H], BF16, isOutput=False)
    w1t_d = nc.declare_dram_parameter("w1t", [50, 400], BF16, isOutput=False)
    w2tp_d = nc.declare_dram_parameter("w2tp", [100, 40], BF16, isOutput=False)
    pm_d = nc.declare_dram_parameter("pmats", [128, 192], BF16, isOutput=False)
    perm_d = nc.declare_dram_parameter("perm", [128, 128], BF16, isOutput=False)
    out_d = nc.declare_dram_parameter("logits", [B_CORE, 10], F32, isOutput=True)

    act = mybir.ActivationFunctionType

    with TileContext(nc) as tc:
        with (
            tc.tile_pool(name="consts", bufs=1) as cpool,
            tc.tile_pool(name="xchunks", bufs=3) as xpool,
            tc.tile_pool(name="relu", bufs=26) as rpool,
            tc.tile_pool(name="small", bufs=2) as spool,
            tc.tile_pool(name="mm1ps", bufs=2, space="PSUM") as mm1pool,
            tc.tile_pool(name="featps", bufs=2, space="PSUM") as featpool,
            tc.tile_pool(name="tailps", bufs=1, space="PSUM") as tailpool,
        ):
            # W1T staged at partition offsets 0 and 64 — the moving operand
            # must share the stationary's base partition (array row offset).
            w1t = cpool.tile([128, 400], BF16)
            w2tp = cpool.tile([100, 40], BF16)
            pmats = cpool.tile([128, 192], BF16)
            perm = cpool.tile([128, 128], BF16)
            nc.sync.dma_start(out=w1t[0:50, :], in_=w1t_d[:])
            nc.sync.dma_start(out=w1t[64:114, :], in_=w1t_d[:])
            nc.sync.dma_start(out=w2tp[:], in_=w2tp_d[:])
            nc.sync.dma_start(out=pmats[:], in_=pm_d[:])
            nc.sync.dma_start(out=perm[:], in_=perm_d[:])

            evac_ct = 0
            for G in [g for _ in range(repeat) for g in range(NG)]:
                xc = xpool.tile([128, CHUNK], BF16, name="xc")
                nc.sync.dma_start(
                    out=xc[:], in_=xp_d[:, G * CHUNK : (G + 1) * CHUNK]
                )

                # ---- Phase A: matmul1 + relu evacuation (48 tiles) ----
                pairs = []
                ps = None
                for g8 in range(8):
                    for j_ in range(3):
                        tcol = (g8 * 3 + j_) * 128
                        for hh in range(2):
                            if hh == 0:
                                ps = mm1pool.tile([128, 1024], F32, name="ps")
                            rb = 64 * hh
                            nc.tensor.matmul(
                                ps[:, 512 * hh : 512 * hh + 400],
                                xc[rb : rb + 50, tcol : tcol + 128],
                                w1t[rb : rb + 50, :],
                                start=True,
                                stop=True,
                            )
                        # evacuate the pair (both halves) in one op
                        rl = rpool.tile(
                            [128, 2, 400], BF16, name="rl", bufs=26
                        )
                        src = ps[:, :].rearrange("p (two k) -> p two k", two=2)[
                            :, :, 0:400
                        ]
                        # measured: ACT 767ns vs DVE 850ns per pair -> 13:11
                        if evac_ct % 24 in (0, 2, 4, 6, 9, 11, 13, 15, 17, 19, 21):
                            nc.vector.tensor_scalar_max(rl[:], src, 0.0)
                        else:
                            nc.scalar.activation(rl[:], src, act.Relu)
                        evac_ct += 1
                        pairs.append(rl)

                # ---- Phase B: pooling matmuls (PE, 4 col-strips) ----
                featps = [
                    featpool.tile([128, 400], F32, name="featps") for _ in range(2)
                ]
                for r in range(3):
                    for q in range(2):
                        for hh in range(2):
                            for s in range(4):
                                g8 = 4 * q + s
                                rl = pairs[g8 * 3 + r]
                                first = r == 0 and q == 0
                                last = r == 2 and q == 1
                                nc.tensor.matmul(
                                    featps[hh][32 * s : 32 * s + 32, :],
                                    pmats[:, (r * 2 + q) * 32 : (r * 2 + q) * 32 + 32],
                                    rl[:, hh, :],
                                    start=first,
                                    stop=last,
                                    # auto-derive rejects base partition 96
                                    tile_position=(0, 32 * s),
                                )

                # ---- Tail per half: feat evac, transpose, matmul2, out ----
                for hh in range(2):
                    feat = spool.tile([128, 400], BF16, name="feat")
                    nc.scalar.activation(feat[:], featps[hh][:], act.Relu)
                    # feat >= 0 so Relu == Copy; Relu keeps bias-free form.

                    ftps = tailpool.tile([100, 512], BF16, name="ftps")
                    for c in range(4):
                        nc.tensor.matmul(
                            ftps[:, 128 * c : 128 * (c + 1)],
                            feat[:, 100 * c : 100 * (c + 1)],
                            perm[:],
                            is_transpose=True,
                            start=(c == 0),
                            stop=(c == 3),
                        )
                    ft = spool.tile([100, 512], BF16, name="ft")
                    nc.vector.tensor_copy(out=ft[:], in_=ftps[:])

                    lps = tailpool.tile([128, 16], F32, name="lps")
                    for c in range(4):
                        nc.tensor.matmul(
                            lps[:, 0:10],
                            ft[:, 128 * c : 128 * (c + 1)],
                            w2tp[:, 10 * c : 10 * (c + 1)],
                            start=(c == 0),
                            stop=(c == 3),
                        )
                    lsb = spool.tile([128, 10], F32, name="lsb")
                    nc.scalar.copy(lsb[:], lps[:, 0:10])
                    rowbase = hh * (B_CORE // 2) + G * 128
                    nc.sync.dma_start(
                        out=out_d[rowbase : rowbase + 128, :], in_=lsb[:]
                    )
    return nc


_NC = {}


def _get_nc(repeat: int = 1):
    if repeat not in _NC:
        _NC[repeat] = _build_nc(repeat)
    return _NC[repeat]


def prepare_in_maps(x: np.ndarray, W1: np.ndarray, W2: np.ndarray):
    assert x.shape == (T, N) and W1.shape == (K, N) and W2.shape == (C, K)

    w1t = np.ascontiguousarray(W1.T.astype(nbf))          # [50, 400]
    w2tp = np.ascontiguousarray(
        (W2.T.astype(np.float32) / J).reshape(4, 100, 10).transpose(1, 0, 2)
        .reshape(100, 40)
    ).astype(nbf)                                          # [100, 4*10]
    pmats = _build_pmats()
    permm = _build_permmat()

    xb = x.astype(nbf)
    in_maps = []
    for c in range(NCORES):
        shard = xb[c * TC : (c + 1) * TC]                  # [98304, 50]
        xp = np.zeros((128, H), nbf)
        xp[0:50] = shard[0:H].T
        xp[64:114] = shard[H:].T
        in_maps.append(
            {
                "xp": xp,
                "w1t": w1t,
                "w2tp": w2tp,
                "pmats": pmats,
                "perm": permm,
            }
        )
    return in_maps


def kernel(x: np.ndarray, W1: np.ndarray, W2: np.ndarray) -> np.ndarray:
    in_maps = prepare_in_maps(x, W1, W2)
    nc = _get_nc()
    res = run_bass_kernel_spmd(nc, in_maps, core_ids=list(range(NCORES)))
    out = np.concatenate(
        [res.results[c]["logits"] for c in range(NCORES)], axis=0
    )
    return out.astype(np.float32)

